# revision 12
# baseline (speedup 1.0000x reference)
"""Tucker-style 3-mode contraction kernel for Trainium2 (8 NeuronCores).

Problem: x [1024*32*32*32] fp32, w0/w1/w2 [32,32] fp32.
  out[B,A,Bb,C] = sum_{a,b,c} x[B,a,b,c] w0[a,A] w1[b,Bb] w2[c,C]

Data-parallel over batch: 128 batch elems/core; sub-tile = 4 batch elems
("groups" g) x full 32x32x32 tensor -> [128 p = (g, mode), 1024 f].
Stationary weights are kron(I4, w) [128,128]; one matmul (2x N=512)
contracts the partition-inner mode of all 4 groups at once.

v3: fp16 intermediates; the three DVE StreamTransposes run on
uint32-PACKED data (fp16 pairs along a bystander mode) so each ST is
[128,512] instead of [128,1024] (halved DVE time). The psum drains
double as free-dim reorders that maintain the packing; work is spread
ACT / DVE / GPSIMD so every engine sits just under the DMA roofline.

  X    [(g,a),(b,c)] f32       <- dense DMA in (super-tile 2 MiB)
  C0   cvt f16, pack b-pairs  -> xb  [p, b2, c, bi]     (Pool + DVE)
  T0   ST u32 [128,512]       -> xt  [(g,c), (b2,a,bi)]
  MM1  kron(w2) f16           -> z1  [(g,C), (b2,a,bi)] psum f32
  D1   ACT reorder+cvt        -> z1b [p, a2, b, ai]
  T1   ST u32                 -> z1t [(g,b), (a2,C,ai)]
  MM2  kron(w1) f16           -> z2  [(g,B), (a2,C,ai)] psum f32
  D2   ACT reorder+cvt        -> z2b [p, C2, a, Ci]
  T2   ST u32                 -> z2t [(g,a), (C2,B,Ci)]
  MM3  kron(w0) f16           -> z3  [(g,A), (C2,B,Ci)] psum f32
  OUT  reorder -> Y [(g,A),(B,C)] f32 (ACT + DVE) -> dense DMA out
"""

import os

import numpy as np

N_CORES = 8
BATCH = 1024
F = 32  # factor dim
ELEM = F * F * F  # 32768 elems per batch element
B_PER_CORE = BATCH // N_CORES  # 128
G = 4  # batch groups per sub-tile (4*32 = 128 partitions)
S = 4  # sub-tiles per super-tile
T = B_PER_CORE // (G * S)  # 8 super-tiles per core
FF = F * F  # 1024
H = F // 2  # 16 pair-count

# intermediate/matmul dtype: "float16" (default) or "bfloat16"
Z_DTYPE = os.environ.get("KERNEL_Z_DTYPE", "float16")
X_DTYPE = Z_DTYPE  # kept for test.py printout compat
# engine split knobs (out of 16 b2-cols for C0; out of 32 b-rows for OUT)
C0_POOL = int(os.environ.get("KERNEL_C0_POOL", "10"))
OUT_ACT = int(os.environ.get("KERNEL_OUT_ACT", "22"))

_CACHE = {}


def build_program(z_dtype=Z_DTYPE, repeat=1):
    key = (z_dtype, repeat)
    if key in _CACHE:
        return _CACHE[key]

    import concourse.bacc as bacc
    import concourse.mybir as mybir
    import concourse.tile as tile

    f32 = mybir.dt.float32
    u32 = mybir.dt.uint32
    zdt = getattr(mybir.dt, z_dtype)

    nc = bacc.Bacc("TRN2", target_bir_lowering=False, debug=False,
                   num_devices=N_CORES)

    xs = nc.dram_tensor("xs", [T, S, G, F, FF], f32, kind="ExternalInput")
    wk2 = nc.dram_tensor("wk2", [128, 128], zdt, kind="ExternalInput")
    wk1 = nc.dram_tensor("wk1", [128, 128], zdt, kind="ExternalInput")
    wk0 = nc.dram_tensor("wk0", [128, 128], zdt, kind="ExternalInput")
    ys = nc.dram_tensor("ys", [T, S, G, F, FF], f32, kind="ExternalOutput")

    def mm(out_ap, lhsT_ap, rhs_ap):
        nc.tensor.matmul(out_ap, lhsT_ap, rhs_ap, start=True, stop=True)

    with tile.TileContext(nc) as tc:
        with (
            tc.tile_pool(name="consts", bufs=1) as cpool,
            tc.tile_pool(name="xp", bufs=2) as xp,
            tc.tile_pool(name="xbp", bufs=2) as xbp,
            tc.tile_pool(name="xtp", bufs=2) as xtp,
            tc.tile_pool(name="z1bp", bufs=2) as z1bp,
            tc.tile_pool(name="z1tp", bufs=2) as z1tp,
            tc.tile_pool(name="z2bp", bufs=2) as z2bp,
            tc.tile_pool(name="z2tp", bufs=2) as z2tp,
            tc.tile_pool(name="yp", bufs=2) as yp,
            tc.tile_pool(name="ps1", bufs=2, space="PSUM") as ps1,
            tc.tile_pool(name="ps2", bufs=1, space="PSUM") as ps2,
            tc.tile_pool(name="ps3", bufs=1, space="PSUM") as ps3,
        ):
            wk2t = cpool.tile([128, 128], zdt)
            wk1t = cpool.tile([128, 128], zdt)
            wk0t = cpool.tile([128, 128], zdt)
            nc.sync.dma_start(out=wk2t[:], in_=wk2[:])
            nc.sync.dma_start(out=wk1t[:], in_=wk1[:])
            nc.sync.dma_start(out=wk0t[:], in_=wk0[:])

            for t in range(T * repeat):
                t = t % T
                X = xp.tile([128, S, FF], f32)  # [(g,a), s, (b,c)]
                nc.sync.dma_start(
                    out=X[:], in_=xs[t].rearrange("s g a m -> (g a) s m"))
                Y = yp.tile([128, S, F, F], f32)  # [(g,A), s, B, C]
                for s in range(S):
                    # C0: cvt f16, pack b-pairs -> xb [p, b2, c, bi]
                    xb = xbp.tile([128, H, F, 2], zdt, tag="xb")
                    xv = X[:, s].rearrange(
                        "p (b2 bi c) -> p b2 c bi", b2=H, bi=2, c=F)
                    nc.gpsimd.tensor_copy(
                        out=xb[:, 0:C0_POOL], in_=xv[:, 0:C0_POOL])
                    nc.vector.tensor_copy(
                        out=xb[:, C0_POOL:H], in_=xv[:, C0_POOL:H])
                    # T0: packed ST -> xt [(g,c), (b2, a)] u32
                    xt = xtp.tile([128, 512], u32, tag="xt")
                    nc.vector.transpose(
                        out=xt[:],
                        in_=xb[:].rearrange("p b2 c bi -> p (b2 c bi)")
                        .bitcast(u32))
                    # MM1: contract c -> z1 [(g,C), (b2,a,bi)]
                    xtv = xt[:].bitcast(zdt)  # [128, 1024] f16
                    z1 = ps1.tile([128, FF], f32, tag="z1")
                    mm(z1[:, 0:512], wk2t[:], xtv[:, 0:512])
                    mm(z1[:, 512:1024], wk2t[:], xtv[:, 512:1024])
                    # D1: reorder+cvt -> z1b [p, a2, (b2,bi)=b, ai]
                    # (split by bi: ACT ISA allows only 3D free patterns)
                    z1b = z1bp.tile([128, H, H, 2, 2], zdt, tag="z1b")
                    z1v = z1[:].rearrange("p (m bi) -> p m bi", m=512, bi=2)
                    for bi in range(2):
                        nc.scalar.copy(
                            out=z1b[:, :, :, bi, :],
                            in_=z1v[:, :, bi].rearrange(
                                "p (b2 a2 ai) -> p a2 b2 ai",
                                b2=H, a2=H, ai=2))
                    # T1: packed ST -> z1t [(g,b), (a2, C)] u32
                    z1t = z1tp.tile([128, 512], u32, tag="z1t")
                    nc.vector.transpose(
                        out=z1t[:],
                        in_=z1b[:].rearrange("p a2 b2 bi ai -> p (a2 b2 bi ai)")
                        .bitcast(u32))
                    # MM2: contract b -> z2 [(g,B), (a2,C,ai)]
                    z1tv = z1t[:].bitcast(zdt)
                    z2 = ps2.tile([128, FF], f32, tag="z2")
                    mm(z2[:, 0:512], wk1t[:], z1tv[:, 0:512])
                    mm(z2[:, 512:1024], wk1t[:], z1tv[:, 512:1024])
                    # D2: reorder+cvt -> z2b [p, C2, (a2,ai)=a, Ci]
                    z2b = z2bp.tile([128, H, H, 2, 2], zdt, tag="z2b")
                    z2v = z2[:].rearrange("p (m ai) -> p m ai", m=512, ai=2)
                    for ai in range(2):
                        nc.scalar.copy(
                            out=z2b[:, :, :, ai, :],
                            in_=z2v[:, :, ai].rearrange(
                                "p (a2 c2 ci) -> p c2 a2 ci",
                                a2=H, c2=H, ci=2))
                    # T2: packed ST -> z2t [(g,a), (C2, B)] u32
                    z2t = z2tp.tile([128, 512], u32, tag="z2t")
                    nc.vector.transpose(
                        out=z2t[:],
                        in_=z2b[:].rearrange("p c2 a2 ai ci -> p (c2 a2 ai ci)")
                        .bitcast(u32))
                    # MM3: contract a -> z3 [(g,A), (C2,B,Ci)]
                    z2tv = z2t[:].bitcast(zdt)
                    z3 = ps3.tile([128, FF], f32, tag="z3")
                    mm(z3[:, 0:512], wk0t[:], z2tv[:, 0:512])
                    mm(z3[:, 512:1024], wk0t[:], z2tv[:, 512:1024])
                    # OUT: reorder -> Y [p, B, (C2,Ci)=C] f32 (split ACT/DVE)
                    zv = z3[:].rearrange(
                        "p (c2 b ci) -> p b c2 ci", c2=H, b=F, ci=2)
                    yv = Y[:, s].rearrange(
                        "p b (c2 ci) -> p b c2 ci", c2=H, ci=2)
                    nc.scalar.copy(out=yv[:, 0:OUT_ACT], in_=zv[:, 0:OUT_ACT])
                    nc.vector.tensor_copy(
                        out=yv[:, OUT_ACT:F], in_=zv[:, OUT_ACT:F])
                nc.scalar.dma_start(
                    out=ys[t].rearrange("s g a (b c) -> (g a) s b c", b=F, c=F),
                    in_=Y[:])

    nc.compile()
    _CACHE[key] = nc
    return nc


def _kron4(w, np_dtype):
    return np.kron(np.eye(G, dtype=np.float32),
                   np.asarray(w, np.float32)).astype(np_dtype)


def make_in_maps(x, w0, w1, w2, z_dtype=Z_DTYPE):
    import ml_dtypes
    zdt_np = np.dtype(ml_dtypes.bfloat16) if z_dtype == "bfloat16" \
        else np.dtype(np.float16)
    x = np.ascontiguousarray(np.asarray(x, np.float32).reshape(-1))
    assert x.size == BATCH * ELEM
    shards = x.reshape(N_CORES, T, S, G, F, FF)
    wk2 = _kron4(w2, zdt_np)
    wk1 = _kron4(w1, zdt_np)
    wk0 = _kron4(w0, zdt_np)
    return [
        {"xs": shards[i], "wk2": wk2, "wk1": wk1, "wk0": wk0}
        for i in range(N_CORES)
    ]


def kernel(x, w0, w1, w2, trace=False):
    from concourse.bass_utils import run_bass_kernel_spmd

    nc = build_program()
    in_maps = make_in_maps(x, w0, w1, w2)
    res = run_bass_kernel_spmd(nc, in_maps, core_ids=list(range(N_CORES)),
                               trace=trace)
    out = np.concatenate([res.results[i]["ys"].reshape(-1)
                          for i in range(N_CORES)])
    if trace:
        return out, res
    return out


# revision 13
# speedup vs baseline: 1.2974x; 1.2974x over previous
"""Tucker-style 3-mode contraction kernel for Trainium2 (8 NeuronCores).

Problem: x [1024*32*32*32] fp32, w0/w1/w2 [32,32] fp32.
  out[B,A,Bb,C] = sum_{a,b,c} x[B,a,b,c] w0[a,A] w1[b,Bb] w2[c,C]

Data-parallel over batch: 128 batch elems/core; sub-tile = 4 batch elems
("groups" g) x full 32x32x32 tensor -> [128 p = (g, mode), 1024 f].
Stationary weights are kron(I4, w) [128,128]; one matmul (2x N=512)
contracts the partition-inner mode of all 4 groups at once.

v4 (contract order c, b, a; fp16 intermediates; all engine access
patterns dense or large-run strided; work spread so no engine exceeds
the DMA roofline):

  X    [(g,a),(b,c)] f32      <- dense DMA in (super-tile 2 MiB)
  C0   cast f16 (dense)      -> xbd [p,(b,c)]      (Pool cols + ACT cols)
  T0   DVE ST                -> xt  [(g,c),(b,a)]
  MM1  kron(w2) f16          -> z1  [(g,C),(b,a)]  psum f32
  D1   ACT reorder+cast      -> z1b [p,(a,b)] f16
  T1   DVE ST                -> z1t [(g,b),(a,C)]
  MM2  kron(w1) f16          -> z2  [(g,B),(a,C)]  psum f32
  D2   ACT reorder+cast      -> z2b [p,(C,a)] f16
  T2   DVE ST                -> z2t [(g,a),(C,B)]
  MM3  kron(w0) f16          -> z3  [(g,A),(C,B)]  psum f32
  OUT  reorder (C,B)->(B,C)  -> Y f32 (ACT rows + DVE rows) -> DMA out
"""

import os

import numpy as np

N_CORES = 8
BATCH = 1024
F = 32  # factor dim
ELEM = F * F * F  # 32768 elems per batch element
B_PER_CORE = BATCH // N_CORES  # 128
G = 4  # batch groups per sub-tile (4*32 = 128 partitions)
S = 4  # sub-tiles per super-tile
T = B_PER_CORE // (G * S)  # 8 super-tiles per core
FF = F * F  # 1024

# intermediate/matmul dtype: "float16" (default) or "bfloat16"
Z_DTYPE = os.environ.get("KERNEL_Z_DTYPE", "float16")
X_DTYPE = Z_DTYPE  # kept for test.py printout compat
# engine split knobs
C0_POOL = int(os.environ.get("KERNEL_C0_POOL", "720"))  # of 1024 cols
OUT_ACT = int(os.environ.get("KERNEL_OUT_ACT", "18"))   # of 32 B-rows

_CACHE = {}


def build_program(z_dtype=Z_DTYPE, repeat=1):
    key = (z_dtype, repeat)
    if key in _CACHE:
        return _CACHE[key]

    import concourse.bacc as bacc
    import concourse.mybir as mybir
    import concourse.tile as tile

    f32 = mybir.dt.float32
    zdt = getattr(mybir.dt, z_dtype)

    nc = bacc.Bacc("TRN2", target_bir_lowering=False, debug=False,
                   num_devices=N_CORES)

    xs = nc.dram_tensor("xs", [T, S, G, F, FF], f32, kind="ExternalInput")
    wk2 = nc.dram_tensor("wk2", [128, 128], zdt, kind="ExternalInput")
    wk1 = nc.dram_tensor("wk1", [128, 128], zdt, kind="ExternalInput")
    wk0 = nc.dram_tensor("wk0", [128, 128], zdt, kind="ExternalInput")
    ys = nc.dram_tensor("ys", [T, S, G, F, FF], f32, kind="ExternalOutput")

    def mm(out_ap, lhsT_ap, rhs_ap):
        nc.tensor.matmul(out_ap, lhsT_ap, rhs_ap, start=True, stop=True)

    with tile.TileContext(nc) as tc:
        with (
            tc.tile_pool(name="consts", bufs=1) as cpool,
            tc.tile_pool(name="xp", bufs=2) as xp,
            tc.tile_pool(name="xbp", bufs=3) as xbp,
            tc.tile_pool(name="xtp", bufs=3) as xtp,
            tc.tile_pool(name="z1bp", bufs=3) as z1bp,
            tc.tile_pool(name="z1tp", bufs=3) as z1tp,
            tc.tile_pool(name="z2bp", bufs=3) as z2bp,
            tc.tile_pool(name="z2tp", bufs=3) as z2tp,
            tc.tile_pool(name="yp", bufs=2) as yp,
            tc.tile_pool(name="ps1", bufs=2, space="PSUM") as ps1,
            tc.tile_pool(name="ps2", bufs=1, space="PSUM") as ps2,
            tc.tile_pool(name="ps3", bufs=1, space="PSUM") as ps3,
        ):
            wk2t = cpool.tile([128, 128], zdt)
            wk1t = cpool.tile([128, 128], zdt)
            wk0t = cpool.tile([128, 128], zdt)
            nc.sync.dma_start(out=wk2t[:], in_=wk2[:])
            nc.sync.dma_start(out=wk1t[:], in_=wk1[:])
            nc.sync.dma_start(out=wk0t[:], in_=wk0[:])

            for t in range(T * repeat):
                t = t % T
                X = xp.tile([128, S, FF], f32)  # [(g,a), s, (b,c)]
                nc.sync.dma_start(
                    out=X[:], in_=xs[t].rearrange("s g a m -> (g a) s m"))
                Y = yp.tile([128, S, F, F], f32)  # [(g,A), s, B, C]
                for s in range(S):
                    # C0: dense cast f32 -> f16 (Pool front cols, ACT rest)
                    xbd = xbp.tile([128, FF], zdt, tag="xbd")
                    nc.gpsimd.tensor_copy(
                        out=xbd[:, 0:C0_POOL], in_=X[:, s, 0:C0_POOL])
                    nc.scalar.copy(
                        out=xbd[:, C0_POOL:FF], in_=X[:, s, C0_POOL:FF])
                    # T0: [(g,a),(b,c)] -> [(g,c),(b,a)]
                    xt = xtp.tile([128, FF], zdt, tag="xt")
                    nc.vector.transpose(out=xt[:], in_=xbd[:])
                    # MM1: contract c -> z1 [(g,C),(b,a)]
                    z1 = ps1.tile([128, FF], f32, tag="z1")
                    mm(z1[:, 0:512], wk2t[:], xt[:, 0:512])
                    mm(z1[:, 512:1024], wk2t[:], xt[:, 512:1024])
                    # D1: reorder (b,a)->(a,b) + cast -> z1b [p,(a,b)]
                    z1b = z1bp.tile([128, F, F], zdt, tag="z1b")
                    nc.scalar.copy(
                        out=z1b[:],
                        in_=z1[:].rearrange("p (b a) -> p a b", b=F, a=F))
                    # T1: -> z1t [(g,b),(a,C)]
                    z1t = z1tp.tile([128, FF], zdt, tag="z1t")
                    nc.vector.transpose(
                        out=z1t[:], in_=z1b[:].rearrange("p a b -> p (a b)"))
                    # MM2: contract b -> z2 [(g,B),(a,C)]
                    z2 = ps2.tile([128, FF], f32, tag="z2")
                    mm(z2[:, 0:512], wk1t[:], z1t[:, 0:512])
                    mm(z2[:, 512:1024], wk1t[:], z1t[:, 512:1024])
                    # D2: reorder (a,C)->(C,a) + cast -> z2b [p,(C,a)]
                    z2b = z2bp.tile([128, F, F], zdt, tag="z2b")
                    nc.scalar.copy(
                        out=z2b[:],
                        in_=z2[:].rearrange("p (a c) -> p c a", a=F, c=F))
                    # T2: -> z2t [(g,a),(C,B)]
                    z2t = z2tp.tile([128, FF], zdt, tag="z2t")
                    nc.vector.transpose(
                        out=z2t[:], in_=z2b[:].rearrange("p c a -> p (c a)"))
                    # MM3: contract a -> z3 [(g,A),(C,B)]
                    z3 = ps3.tile([128, FF], f32, tag="z3")
                    mm(z3[:, 0:512], wk0t[:], z2t[:, 0:512])
                    mm(z3[:, 512:1024], wk0t[:], z2t[:, 512:1024])
                    # OUT: reorder (C,B)->(B,C) -> Y f32 (ACT rows + DVE rows)
                    zv = z3[:].rearrange("p (c b) -> p b c", c=F, b=F)
                    nc.scalar.copy(
                        out=Y[:, s, 0:OUT_ACT], in_=zv[:, 0:OUT_ACT])
                    nc.vector.tensor_copy(
                        out=Y[:, s, OUT_ACT:F], in_=zv[:, OUT_ACT:F])
                nc.scalar.dma_start(
                    out=ys[t].rearrange("s g a (b c) -> (g a) s b c", b=F, c=F),
                    in_=Y[:])

    nc.compile()
    _CACHE[key] = nc
    return nc


def _kron4(w, np_dtype):
    return np.kron(np.eye(G, dtype=np.float32),
                   np.asarray(w, np.float32)).astype(np_dtype)


def make_in_maps(x, w0, w1, w2, z_dtype=Z_DTYPE):
    import ml_dtypes
    zdt_np = np.dtype(ml_dtypes.bfloat16) if z_dtype == "bfloat16" \
        else np.dtype(np.float16)
    x = np.ascontiguousarray(np.asarray(x, np.float32).reshape(-1))
    assert x.size == BATCH * ELEM
    shards = x.reshape(N_CORES, T, S, G, F, FF)
    wk2 = _kron4(w2, zdt_np)
    wk1 = _kron4(w1, zdt_np)
    wk0 = _kron4(w0, zdt_np)
    return [
        {"xs": shards[i], "wk2": wk2, "wk1": wk1, "wk0": wk0}
        for i in range(N_CORES)
    ]


def kernel(x, w0, w1, w2, trace=False):
    from concourse.bass_utils import run_bass_kernel_spmd

    nc = build_program()
    in_maps = make_in_maps(x, w0, w1, w2)
    res = run_bass_kernel_spmd(nc, in_maps, core_ids=list(range(N_CORES)),
                               trace=trace)
    out = np.concatenate([res.results[i]["ys"].reshape(-1)
                          for i in range(N_CORES)])
    if trace:
        return out, res
    return out
